# revision 1
# baseline (speedup 1.0000x reference)
"""Trainium2 Bass kernel for the circular drift-diffusion loss (batched expm).

Reference computes  loss = -mean_b log(relu(e_{idx_b}^T expm(t_b*A) p0_b) + eps)
with A a fixed 360x360 circular advection-diffusion operator, t_b in [0,1000),
p0_b a von Mises density, over a batch of 256.

Algorithm (per core; batch sharded 32/core over 8 cores):
  * Quantize t_b = m_b*T0 + r_b with T0 = 1000/2^K, m_b < 2^K.
  * Build propagator chain M_j = expm(2^j*T0*A) once by repeated squaring
    (prelude: ascending Taylor at T0/2^PRE_SQ, then PRE_SQ squarings -> M_0;
    then K-1 squarings).  A squaring is 9 f32 matmuls for S = M@M plus 9 PE
    transposes for S^T (needed as the next stationary operand).  K and the
    Taylor degrees are chosen at runtime from ||A||_inf so both
    heavy-diffusion and near-advection inputs are optimal.
  * Apply bits of m_b as masked batched matvecs: Q <- bit_j ? M_j Q : Q.
  * Residual: Q <- Taylor_DEG_R(r_b A) Q (Horner, per-sample scalar folded
    into host-precomputed r/k coefficient tables).
  * p0 built on device (folded poly cos + Exp activation), selection via
    one-hot + PE column-sum, loss terms via Ln activation.
Everything O(n^2)+ runs on device; host does only index/bit/layout glue and
the tridiagonal operator assembly (exactly replicating the reference's f32
evo_mat construction).
"""

import math

import numpy as np

# ---------------- static problem constants (hardcoded per contract) ----------
N = 360            # color mesh size
P = 120            # partition chunk (N = 3*P)
NCH = 3            # chunks
B = 256            # total batch
NCORES = 8
BL = B // NCORES   # per-core batch
T_MAX = 1000.0
KAPPA = 400.0      # 1/SIGMA_INIT^2
EPS = 1e-5
TWO_PI = 6.283185307179586
# ln(1/(2*pi*i0e(400)))  [i0e(400) = 0.019953356281939987]
LNC = 2.076480848703078
# cos(sqrt(u)) on u in [0, pi^2] (|delta| folded to [0,pi]), power basis c0..c8
COS_COEF = [1.00000000e+00, -5.00000000e-01, 4.16666666e-02, -1.38888885e-03,
            2.48015646e-05, -2.75566515e-07, 2.08651966e-09, -1.13535474e-11,
            4.13131734e-14]

_COMPILED = {}


def _taylor_deg(x, tol, lo):
    """Smallest d with x^(d+1)/(d+1)! < tol."""
    d = lo
    term = x ** (d + 1) / math.factorial(d + 1)
    while term > tol and d < 40:
        d += 1
        term *= x / (d + 1)
    return d


def _plan(anorm):
    """Choose (k_bits, deg_p, deg_r) from ||A||_inf.  The time grid is
    T0 = T_MAX/2^k_bits, chosen so the prelude Taylor converges fast at T0;
    every squaring level applies one bit of the quantized delay."""
    xa = T_MAX * float(anorm)
    if xa <= 0.0:
        return 2, 4, 3
    k0 = max(2, min(16, math.ceil(math.log2(max(xa / 0.9, 2.0)))))

    def degrees(k):
        x0 = xa / (1 << k)
        # prelude truncation amplifies roughly 2^(k/2) through the
        # squarings, so its tolerance adapts to chain depth; the residual
        # Taylor is applied once (no amplification) and tolerates more.
        tol_p = min(max(3e-5 / 2 ** (k / 2), 5e-8), 2e-6)
        return _taylor_deg(x0, tol_p, 4), _taylor_deg(x0, 1e-6, 3)

    # pick k by explicit cost minimization with measured per-stage costs
    # (chain level 7.6us, prelude step 4.5us, taylor step 0.7us)
    best = None
    for k in range(max(2, k0 - 1), min(16, k0 + 2) + 1):
        dp, dr = degrees(k)
        cost = (k - 1) * 7.6 + (dp - 1) * 4.5 + dr * 0.7
        if best is None or cost < best[0]:
            best = (cost, k, dp, dr)
    _, k, deg_p, deg_r = best
    return k, deg_p, deg_r


def _build_bass(k_bits, deg_p, deg_r):
    """Construct the Bass program (SPMD; identical on all 8 cores)."""
    import concourse.tile as tile
    from concourse import bacc, mybir

    F32 = mybir.dt.float32
    AF = mybir.ActivationFunctionType
    OP = mybir.AluOpType

    nc = bacc.Bacc("TRN2", target_bir_lowering=False, debug=False)

    def din(name, shape, dt=F32):
        return nc.dram_tensor(name, shape, dt, kind="ExternalInput").ap()

    d_x = din("x", [N, N])        # X = T0*A
    d_xt = din("xt", [N, N])      # X^T
    d_cm = din("cm", [P, NCH])    # c_mesh chunks: cm[p,c] = c_mesh[c*P+p]
    d_irep = din("irep", [P, BL])            # init_color replicated
    d_msk = din("msk", [P, k_bits * BL], mybir.dt.uint8)  # bit masks (0/1)
    d_rdk = din("rdk", [P, deg_r * BL])      # (r/T0)/k, k=1..deg_r
    d_oh = din("oh", [P, NCH * BL])          # one-hot selector chunks
    d_out = nc.dram_tensor("terms", [1, BL], F32, kind="ExternalOutput").ap()

    with tile.TileContext(nc) as tc:
        with (
            tc.tile_pool(name="const", bufs=1) as cpool,
            tc.tile_pool(name="mats", bufs=3) as mpool,
            tc.tile_pool(name="qp", bufs=2) as qpool,
            tc.tile_pool(name="vp", bufs=3) as vpool,
            tc.tile_pool(name="tp", bufs=4) as tpool,
            tc.tile_pool(name="psb", bufs=5, space="PSUM") as psb,
            tc.tile_pool(name="pss", bufs=3, space="PSUM") as pss,
        ):
            # ---- constants ------------------------------------------------
            # chunk 0 of x/xt lands as [P,P] pieces on parallel queues so
            # the first prelude matmul starts early; other chunks whole
            XN = cpool.tile([P, NCH * N], F32, tag="x")
            XT = cpool.tile([P, NCH * N], F32, tag="xt")
            # chunk 0 of x lands as [P,P] pieces plus the first XT block so
            # the first prelude matmuls start early; other chunks whole
            for b in range(NCH):
                nc.sync.dma_start(XN[:, b * P:(b + 1) * P],
                                  d_x[0:P, b * P:(b + 1) * P])
            nc.sync.dma_start(XT[:, 0:P], d_xt[0:P, 0:P])
            for c in range(1, NCH):
                nc.sync.dma_start(XN[:, c * N:(c + 1) * N],
                                  d_x[c * P:(c + 1) * P, :])
                nc.sync.dma_start(XT[:, c * N:(c + 1) * N],
                                  d_xt[c * P:(c + 1) * P, :])
            nc.sync.dma_start(XT[:, P:N], d_xt[0:P, P:N])
            CM = cpool.tile([P, NCH], F32, tag="cm")
            nc.sync.dma_start(CM[:], d_cm[:])
            IREP = cpool.tile([P, BL], F32, tag="irep")
            nc.sync.dma_start(IREP[:], d_irep[:])
            MSK = cpool.tile([P, k_bits * BL], mybir.dt.uint8, tag="msk")
            nc.sync.dma_start(MSK[:], d_msk[:])
            RDK = cpool.tile([P, deg_r * BL], F32, tag="rdk")
            nc.sync.dma_start(RDK[:], d_rdk[:])
            OH = cpool.tile([P, NCH * BL], F32, tag="oh")
            nc.sync.dma_start(OH[:], d_oh[:])
            ONES = cpool.tile([P, 1], F32, tag="ones")
            nc.vector.memset(ONES[:], 1.0)
            BEXP = cpool.tile([P, 1], F32, tag="bexp")
            nc.vector.memset(BEXP[:], LNC - KAPPA)
            BLN0 = cpool.tile([1, 1], F32, tag="bln0")
            nc.vector.memset(BLN0[:], 0.0)
            # identities built on device (no DMA)
            EYE = cpool.tile([P, NCH * N], F32, tag="eye")
            nc.vector.memset(EYE[:], 1.0)
            nc.gpsimd.affine_select(
                EYE[:].rearrange("p (c n) -> p c n", c=NCH),
                EYE[:].rearrange("p (c n) -> p c n", c=NCH),
                pattern=[[-P, NCH], [1, N]], compare_op=OP.is_equal,
                fill=0.0, base=0, channel_multiplier=-1,
            )
            E120 = cpool.tile([P, P], F32, tag="e120")
            nc.vector.memset(E120[:], 1.0)
            nc.gpsimd.affine_select(
                E120[:], E120[:], pattern=[[1, P]], compare_op=OP.is_equal,
                fill=0.0, base=0, channel_multiplier=-1,
            )

            W = N + BL  # merged chunk width: [M_c | Q_c]

            def mm_group(ps, lhsT_tile, rhs_tile, i, rhs_w, rhs_stride=None):
                rs = rhs_w if rhs_stride is None else rhs_stride
                for c in range(NCH):
                    nc.tensor.matmul(
                        ps[:],
                        lhsT=lhsT_tile[:, c * N + i * P: c * N + i * P + P],
                        rhs=rhs_tile[:, c * rs: c * rs + rhs_w],
                        start=(c == 0), stop=(c == NCH - 1),
                    )

            def copy_out(dst_ap, ps, idx, small=False):
                # PSUM->SBUF copies: big 1/4 DVE 3/4 ACT; small (transpose
                # blocks) 1/2-1/2 -- ACT's fixed overhead dominates there
                mod = 2 if small else 4
                if idx % mod == 0:
                    nc.vector.tensor_copy(dst_ap, ps[:])
                else:
                    nc.scalar.copy(dst_ap, ps[:])

            # ---- p0 (von Mises) in Q-layout [P, NCH*BL] -------------------
            Q = qpool.tile([P, NCH * BL], F32, tag="q")
            for c in range(NCH):
                qs = Q[:, c * BL:(c + 1) * BL]
                dl = tpool.tile([P, BL], F32, tag="t0")
                # delta = init - c_mesh  (cos is even, sign irrelevant)
                nc.vector.tensor_scalar(dl[:], IREP[:], CM[:, c:c + 1], None,
                                        op0=OP.subtract)
                ab = tpool.tile([P, BL], F32, tag="t1")
                nc.scalar.activation(ab[:], dl[:], AF.Abs)
                fl = tpool.tile([P, BL], F32, tag="t2")
                nc.vector.tensor_scalar(fl[:], ab[:], -1.0, TWO_PI,
                                        op0=OP.mult, op1=OP.add)
                w = tpool.tile([P, BL], F32, tag="t3")
                nc.vector.tensor_tensor(w[:], ab[:], fl[:], op=OP.min)
                u = tpool.tile([P, BL], F32, tag="t0")
                nc.vector.tensor_tensor(u[:], w[:], w[:], op=OP.mult)
                h = tpool.tile([P, BL], F32, tag="t1")
                nc.vector.tensor_scalar(h[:], u[:], COS_COEF[8], COS_COEF[7],
                                        op0=OP.mult, op1=OP.add)
                heng = nc.gpsimd if c == 1 else nc.vector
                for k in range(6, -1, -1):
                    heng.tensor_tensor(h[:], h[:], u[:], op=OP.mult)
                    heng.tensor_scalar(h[:], h[:], COS_COEF[k], None,
                                       op0=OP.add)
                # p0 = exp(kappa*cos - kappa + lnC)
                nc.scalar.activation(qs, h[:], AF.Exp, bias=BEXP[:],
                                     scale=KAPPA)

            # ---- residual Taylor on p0 (commutes with the bit applies):
            # V = Q + rdk_k*(X V), k=deg_r..1.  Steps are emitted interleaved
            # with the prelude/chain so the small matmuls fill PE bubbles.
            taylor_state = {"V": Q, "k": deg_r, "dst": None}

            def taylor_step():
                k = taylor_state["k"]
                if k < 1:
                    return
                Vc = taylor_state["V"]
                last = k == 1 and taylor_state["dst"] is not None
                Vn = None if last else vpool.tile([P, NCH * BL], F32,
                                                  tag="V")
                for i in range(NCH):
                    ps = pss.tile([P, BL], F32, tag="ap")
                    mm_group(ps, XT, Vc, i, BL)
                    if last:
                        # final step writes straight into the MQ tile's Q
                        # slots -- keeps the install copies off the
                        # prelude->chain critical path
                        vs = taylor_state["dst"][:, i * W + N: (i + 1) * W]
                    else:
                        vs = Vn[:, i * BL:(i + 1) * BL]
                    nc.vector.tensor_tensor(
                        vs, ps[:], RDK[:, (k - 1) * BL: k * BL], op=OP.mult)
                    nc.gpsimd.tensor_tensor(
                        vs, vs, Q[:, i * BL:(i + 1) * BL], op=OP.add)
                taylor_state["V"] = Vn
                taylor_state["k"] = k - 1

            # ---- prelude: ascending Taylor S = I + sum X^k/k! -------------
            # critical path per step: 9 matmuls + 3 scale-copies; the S+=T
            # adds run on gpsimd off the PE path.  M tiles are MQ-shaped
            # ([M_c | Q_c] per chunk) so later bit-applies merge into the
            # squaring matmuls as 32 extra moving columns.
            S = mpool.tile([P, NCH * W], F32, tag="M")
            taylor_state["dst"] = S
            for c in range(NCH):
                nc.vector.tensor_tensor(S[:, c * W: c * W + N],
                                        XN[:, c * N:(c + 1) * N],
                                        EYE[:, c * N:(c + 1) * N], op=OP.add)
            T = XN
            for k in range(2, deg_p + 1):
                Tn = mpool.tile([P, NCH * N], F32, tag="T")
                for i in range(NCH):
                    ps = psb.tile([P, N], F32, tag="sq")
                    mm_group(ps, XT, T, i, N)
                    if i % 2 == 0:
                        nc.vector.tensor_scalar(Tn[:, i * N:(i + 1) * N],
                                                ps[:], 1.0 / k, None,
                                                op0=OP.mult)
                    else:
                        nc.scalar.mul(Tn[:, i * N:(i + 1) * N], ps[:], 1.0 / k)
                    seng = nc.gpsimd if i == 1 else nc.vector
                    seng.tensor_tensor(S[:, i * W: i * W + N],
                                       S[:, i * W: i * W + N],
                                       Tn[:, i * N:(i + 1) * N],
                                       op=OP.add)
                T = Tn
                taylor_step()

            ST = mpool.tile([P, NCH * N], F32, tag="MT")

            def transpose_mq(MTt, Mt):
                k = 0
                for ib in range(NCH):
                    for cp in range(NCH):
                        pst = psb.tile([P, P], F32, tag="sq")
                        nc.tensor.transpose(
                            pst[:], Mt[:, ib * W + cp * P: ib * W + cp * P + P],
                            E120[:],
                        )
                        copy_out(MTt[:, cp * N + ib * P: cp * N + ib * P + P],
                                 pst, k, small=True)
                        k += 1

            transpose_mq(ST, S)
            M, MT = S, ST

            def square(Mc, MTc, bit=None):
                # Sn = Mc@Mc; if bit is not None also compute Mc@Q (merged
                # columns) and blend it into Sn's Q slot under the bit mask.
                Sn = mpool.tile([P, NCH * W], F32, tag="M")
                STn = mpool.tile([P, NCH * N], F32, tag="MT")
                wid = N if bit is None else W
                for i in range(NCH):
                    ps = psb.tile([P, wid], F32, tag="sq")
                    mm_group(ps, MTc, Mc, i, wid, rhs_stride=W)
                    copy_out(Sn[:, i * W: i * W + N], ps[:, :N], i)
                    if bit is not None:
                        nc.gpsimd.tensor_copy(Sn[:, i * W + N: (i + 1) * W],
                                              Mc[:, i * W + N: (i + 1) * W])
                        nc.vector.copy_predicated(
                            Sn[:, i * W + N: (i + 1) * W],
                            MSK[:, bit * BL:(bit + 1) * BL],
                            ps[:, N:W],
                        )
                transpose_mq(STn, Sn)
                return Sn, STn

            # drain remaining taylor steps; the k==1 step lands the
            # evolved p0 directly in the MQ tile's Q slots
            while taylor_state["k"] >= 1:
                taylor_step()

            # ---- merged bit applies + chain squarings ---------------------
            # level j squares M (= expm(2^j T0 A)) and applies bit j of the
            # quantized delay to Q in the same matmul set.  The top TWO bits
            # need no further squaring: bit k-2 is a single apply of M_{k-2}
            # and bit k-1 a double apply (M_{k-1} Q = M_{k-2} (M_{k-2} Q)),
            # which is ~2x cheaper than materializing M_{k-1}.
            for j in range(k_bits - 2):
                M, MT = square(M, MT, bit=j)

            def apply_bit(q_rhs, rhs_stride, rhs_off, bit, blend_src):
                # psum[i] = M @ q ; if bit is not None blend into a fresh
                # Q tile under the bit mask, else return psum tiles
                aps = []
                for i in range(NCH):
                    ps = pss.tile([P, BL], F32, tag="ap")
                    for c in range(NCH):
                        nc.tensor.matmul(
                            ps[:],
                            lhsT=MT[:, c * N + i * P: c * N + i * P + P],
                            rhs=q_rhs[:, c * rhs_stride + rhs_off:
                                      c * rhs_stride + rhs_off + BL],
                            start=(c == 0), stop=(c == NCH - 1),
                        )
                    aps.append(ps)
                if bit is None:
                    return aps
                Qn = qpool.tile([P, NCH * BL], F32, tag="qf")
                for i in range(NCH):
                    nc.gpsimd.tensor_copy(
                        Qn[:, i * BL:(i + 1) * BL],
                        blend_src[i] if isinstance(blend_src, list)
                        else blend_src[:, i * rhs_stride + rhs_off:
                                       i * rhs_stride + rhs_off + BL])
                    nc.vector.copy_predicated(
                        Qn[:, i * BL:(i + 1) * BL],
                        MSK[:, bit * BL:(bit + 1) * BL],
                        aps[i][:],
                    )
                return Qn

            # bit k-2: single apply on the Q slots of the MQ tile
            Qf = apply_bit(M, W, N, k_bits - 2, M)
            # bit k-1: double apply of the same M
            y1ps = apply_bit(Qf, BL, 0, None, None)
            Y1 = vpool.tile([P, NCH * BL], F32, tag="V")
            for i in range(NCH):
                copy_out(Y1[:, i * BL:(i + 1) * BL], y1ps[i], i, small=True)
            V = apply_bit(Y1, BL, 0, k_bits - 1, Qf)

            # ---- selection + loss terms -----------------------------------
            sel = psb.tile([1, BL], F32, tag="sq")
            for c in range(NCH):
                tmp = tpool.tile([P, BL], F32, tag="t2")
                nc.vector.tensor_tensor(
                    tmp[:], V[:, c * BL:(c + 1) * BL],
                    OH[:, c * BL:(c + 1) * BL], op=OP.mult)
                nc.tensor.matmul(sel[:], lhsT=ONES[:], rhs=tmp[:],
                                 start=(c == 0), stop=(c == NCH - 1))
            # ln(relu(psel)+eps) via exponent/mantissa split: the HW Ln
            # table degrades for huge args (psel can reach ~1e20 in the
            # weak-diffusion regime), so compute ln(m) + e*ln2 with m in
            # [1,2), which keeps the table in its accurate range.
            I32 = mybir.dt.int32
            rl = tpool.tile([1, BL], F32, tag="r0")
            nc.vector.tensor_scalar(rl[:], sel[:], 0.0, EPS,
                                    op0=OP.max, op1=OP.add)
            xi = rl[:].bitcast(I32)
            et = tpool.tile([1, BL], I32, tag="r2")
            nc.vector.tensor_scalar(et[:], xi, 23, None,
                                    op0=OP.arith_shift_right)
            ef = tpool.tile([1, BL], F32, tag="r3")
            nc.vector.tensor_copy(ef[:], et[:])
            mi = tpool.tile([1, BL], I32, tag="r4")
            nc.vector.tensor_scalar(mi[:], xi, 0x007FFFFF, 0x3F800000,
                                    op0=OP.bitwise_and, op1=OP.bitwise_or)
            lnm = tpool.tile([1, BL], F32, tag="r5")
            nc.scalar.activation(lnm[:], mi[:].bitcast(F32), AF.Ln,
                                 bias=BLN0[:], scale=1.0)
            terms = tpool.tile([1, BL], F32, tag="r1")
            # ef holds the biased exponent; fold the -127*ln2 into the mult
            nc.vector.tensor_scalar(terms[:], ef[:], 0.6931471805599453,
                                    -88.02969193111305,
                                    op0=OP.mult, op1=OP.add)
            nc.vector.tensor_tensor(terms[:], terms[:], lnm[:], op=OP.add)
            nc.sync.dma_start(d_out[:], terms[:])

    nc.compile()
    return nc


def _host_prep(c_mesh, gtheta, sigma_diff, init_color, delay_t, report_color):
    """Host-side glue: operator assembly (replicating reference f32 ops),
    plan selection, and per-core index/bit/layout arrays."""
    f32 = np.float32
    c = np.asarray(c_mesh, dtype=f32)
    g = np.asarray(gtheta, dtype=f32)
    s = np.asarray(sigma_diff, dtype=f32)[0]
    init = np.asarray(init_color, dtype=f32)
    t = np.asarray(delay_t, dtype=f32)
    rep = np.asarray(report_color, dtype=f32)

    d = (c[1] - c[0]).astype(f32)
    eye = np.eye(N, dtype=f32)
    up = np.roll(eye, -1, axis=1)
    dn = np.roll(eye, 1, axis=1)
    D1 = ((up - dn) / (f32(2.0) * d)).astype(f32)
    D2 = ((up - f32(2.0) * eye + dn) / (d * d)).astype(f32)
    A = ((s ** f32(2.0)) / f32(2.0) * D2 - D1 * g[None, :]).astype(f32)

    anorm = np.abs(A.astype(np.float64)).sum(axis=1).max()
    k_bits, deg_p, deg_r = plan = _plan(anorm)
    T0 = T_MAX / (1 << k_bits)
    X = (A * f32(T0)).astype(f32)

    m = np.floor(t.astype(np.float64) / T0).astype(np.int64)
    m = np.clip(m, 0, (1 << k_bits) - 1)
    r = (t.astype(np.float64) - m * T0) / T0  # in X = T0*A units
    bits = ((m[:, None] >> np.arange(k_bits)[None, :]) & 1)     # [B, K]
    idx = np.argmin(np.abs(c[None, :] - rep[:, None]), axis=1)

    shared = {
        "x": X,
        "xt": np.ascontiguousarray(X.T),
        "cm": np.ascontiguousarray(c.reshape(NCH, P).T),
    }
    in_maps = []
    for core in range(NCORES):
        sl = slice(core * BL, (core + 1) * BL)
        irep = np.broadcast_to(init[sl][None, :], (P, BL)).astype(f32)
        msk = np.broadcast_to(
            bits[sl].T.reshape(1, k_bits * BL), (P, k_bits * BL)
        ).astype(np.uint8)  # bit j at [j*BL:(j+1)*BL]
        rdk = np.empty((deg_r, BL), f32)
        for k in range(1, deg_r + 1):
            rdk[k - 1] = (r[sl] / k).astype(f32)
        rdk = np.broadcast_to(
            rdk.reshape(1, deg_r * BL), (P, deg_r * BL)).astype(f32)
        oh = np.zeros((NCH, P, BL), f32)
        for b, ix in enumerate(idx[sl]):
            oh[ix // P, ix % P, b] = 1.0
        oh = np.ascontiguousarray(oh.transpose(1, 0, 2).reshape(P, NCH * BL))
        in_maps.append(dict(shared, irep=irep, msk=msk, rdk=rdk, oh=oh))
    return plan, in_maps


def _get_nc(plan):
    if plan not in _COMPILED:
        _COMPILED[plan] = _build_bass(*plan)
    return _COMPILED[plan]


def kernel(**inputs):
    from concourse.bass_utils import run_bass_kernel_spmd

    plan, in_maps = _host_prep(
        inputs["c_mesh"], inputs["gtheta"], inputs["sigma_diff"],
        inputs["init_color"], inputs["delay_t"], inputs["report_color"],
    )
    nc = _get_nc(plan)
    res = run_bass_kernel_spmd(nc, in_maps, list(range(NCORES)))
    terms = np.concatenate(
        [np.asarray(res.results[k]["terms"]).reshape(-1) for k in range(NCORES)]
    )
    loss = -np.mean(terms.astype(np.float64))
    return np.asarray(loss, dtype=np.float32)



# revision 24
# speedup vs baseline: 1.6157x; 1.6157x over previous
"""Trainium2 Bass kernel for the circular drift-diffusion loss (batched expm).

Reference computes  loss = -mean_b log(relu(e_{idx_b}^T expm(t_b*A) p0_b) + eps)
with A a fixed 360x360 circular advection-diffusion operator, t_b in [0,1000),
p0_b a von Mises density, over a batch of 256.

Algorithm (per core; batch sharded 32/core over 8 cores):
  * Quantize t_b = m_b*T0 + r_b with T0 = 1000/2^K, m_b < 2^K.
  * Build propagator chain M_j = expm(2^j*T0*A) once by repeated squaring
    (prelude: ascending Taylor at T0/2^PRE_SQ, then PRE_SQ squarings -> M_0;
    then K-1 squarings).  A squaring is 9 f32 matmuls for S = M@M plus 9 PE
    transposes for S^T (needed as the next stationary operand).  K and the
    Taylor degrees are chosen at runtime from ||A||_inf so both
    heavy-diffusion and near-advection inputs are optimal.
  * Apply bits of m_b as masked batched matvecs: Q <- bit_j ? M_j Q : Q.
  * Residual: Q <- Taylor_DEG_R(r_b A) Q (Horner, per-sample scalar folded
    into host-precomputed r/k coefficient tables).
  * p0 built on device (folded poly cos + Exp activation), selection via
    one-hot + PE column-sum, loss terms via Ln activation.
Everything O(n^2)+ runs on device; host does only index/bit/layout glue and
the tridiagonal operator assembly (exactly replicating the reference's f32
evo_mat construction).
"""

import math

import numpy as np

# ---------------- static problem constants (hardcoded per contract) ----------
N = 360            # color mesh size
P = 120            # partition chunk (N = 3*P)
NCH = 3            # chunks
B = 256            # total batch
NCORES = 8
BL = B // NCORES   # per-core batch
T_MAX = 1000.0
KAPPA = 400.0      # 1/SIGMA_INIT^2
EPS = 1e-5
TWO_PI = 6.283185307179586
# ln(1/(2*pi*i0e(400)))  [i0e(400) = 0.019953356281939987]
LNC = 2.076480848703078
# cos(sqrt(u)) on u in [0, pi^2] (|delta| folded to [0,pi]), power basis c0..c8
COS_COEF = [1.00000000e+00, -5.00000000e-01, 4.16666666e-02, -1.38888885e-03,
            2.48015646e-05, -2.75566515e-07, 2.08651966e-09, -1.13535474e-11,
            4.13131734e-14]

_COMPILED = {}


def _taylor_deg(x, tol, lo):
    """Smallest d with x^(d+1)/(d+1)! < tol."""
    d = lo
    term = x ** (d + 1) / math.factorial(d + 1)
    while term > tol and d < 40:
        d += 1
        term *= x / (d + 1)
    return d


def _plan(anorm):
    """Choose (k_bits, deg_p, deg_r) from ||A||_inf.  The time grid is
    T0 = T_MAX/2^k_bits, chosen so the prelude Taylor converges fast at T0;
    every squaring level applies one bit of the quantized delay."""
    xa = T_MAX * float(anorm)
    if xa <= 0.0:
        return 2, 4, 3
    k0 = max(2, min(16, math.ceil(math.log2(max(xa / 0.9, 2.0)))))

    def degrees(k):
        x0 = xa / (1 << k)
        # prelude truncation amplifies roughly 2^(k/2) through the
        # squarings, so its tolerance adapts to chain depth; the residual
        # Taylor is applied once (no amplification) and tolerates more.
        tol_p = min(max(3e-5 / 2 ** (k / 2), 5e-8), 2e-6)
        return _taylor_deg(x0, tol_p, 4), _taylor_deg(x0, 1e-6, 3)

    # pick k by explicit cost minimization with measured per-stage costs
    # (chain level 7.6us, prelude step 4.5us, taylor step 0.7us)
    best = None
    for k in range(max(2, k0 - 1), min(16, k0 + 2) + 1):
        dp, dr = degrees(k)
        cost = (k - 1) * 7.6 + (dp - 1) * 4.5 + dr * 0.7
        if best is None or cost < best[0]:
            best = (cost, k, dp, dr)
    _, k, deg_p, deg_r = best
    return k, deg_p, deg_r


def _build_bass(k_bits, deg_p, deg_r):
    """Construct the Bass program (SPMD; identical on all 8 cores)."""
    import concourse.tile as tile
    from concourse import bacc, mybir

    F32 = mybir.dt.float32
    F32R = mybir.dt.float32r
    AF = mybir.ActivationFunctionType
    OP = mybir.AluOpType

    nc = bacc.Bacc("TRN2", target_bir_lowering=False, debug=False)

    def din(name, shape, dt=F32):
        return nc.dram_tensor(name, shape, dt, kind="ExternalInput").ap()

    d_x = din("x", [N, N], F32R)  # X = T0*A
    d_xt = din("xt", [N, N], F32R)  # X^T
    d_cm = din("cm", [P, NCH])    # c_mesh chunks: cm[p,c] = c_mesh[c*P+p]
    d_irep = din("irep", [P, BL])            # init_color replicated
    d_msk = din("msk", [P, k_bits * BL])     # bit masks (0.0/1.0)
    d_rdk = din("rdk", [P, deg_r * BL])      # (r/T0)/k, k=1..deg_r
    d_oh = din("oh", [P, NCH * BL])          # one-hot selector chunks
    d_out = nc.dram_tensor("terms", [1, BL], F32, kind="ExternalOutput").ap()

    with tile.TileContext(nc) as tc:
        with (
            tc.tile_pool(name="const", bufs=1) as cpool,
            tc.tile_pool(name="mats", bufs=3) as mpool,
            tc.tile_pool(name="qp", bufs=2) as qpool,
            tc.tile_pool(name="vp", bufs=3) as vpool,
            tc.tile_pool(name="tp", bufs=4) as tpool,
            tc.tile_pool(name="psb", bufs=5, space="PSUM") as psb,
            tc.tile_pool(name="pss", bufs=3, space="PSUM") as pss,
        ):
            # ---- constants ------------------------------------------------
            # chunk 0 of x/xt lands as [P,P] pieces on parallel queues so
            # the first prelude matmul starts early; other chunks whole
            XN = cpool.tile([P, NCH * N], F32R, tag="x")
            XT = cpool.tile([P, NCH * N], F32R, tag="xt")
            # chunk 0 of x lands as [P,P] pieces plus the first XT block so
            # the first prelude matmuls start early; other chunks whole
            for b in range(NCH):
                nc.sync.dma_start(XN[:, b * P:(b + 1) * P],
                                  d_x[0:P, b * P:(b + 1) * P])
            nc.sync.dma_start(XT[:, 0:P], d_xt[0:P, 0:P])
            for c in range(1, NCH):
                nc.sync.dma_start(XN[:, c * N:(c + 1) * N],
                                  d_x[c * P:(c + 1) * P, :])
                nc.sync.dma_start(XT[:, c * N:(c + 1) * N],
                                  d_xt[c * P:(c + 1) * P, :])
            nc.sync.dma_start(XT[:, P:N], d_xt[0:P, P:N])
            CM = cpool.tile([P, NCH], F32, tag="cm")
            nc.sync.dma_start(CM[:], d_cm[:])
            IREP = cpool.tile([P, BL], F32, tag="irep")
            nc.sync.dma_start(IREP[:], d_irep[:])
            MSK = cpool.tile([P, k_bits * BL], F32, tag="msk")
            nc.sync.dma_start(MSK[:], d_msk[:])
            RDK = cpool.tile([P, deg_r * BL], F32, tag="rdk")
            nc.sync.dma_start(RDK[:], d_rdk[:])
            OH = cpool.tile([P, NCH * BL], F32, tag="oh")
            nc.sync.dma_start(OH[:], d_oh[:])
            # fp32r matmul operands must be produced as fp32r (walrus
            # verifier); memset can't, so constants go via a staging copy
            ONES0 = cpool.tile([P, 1], F32, tag="ones0")
            nc.vector.memset(ONES0[:], 1.0)
            ONES = cpool.tile([P, 1], F32R, tag="ones")
            nc.vector.tensor_copy(ONES[:], ONES0[:])
            BEXP = cpool.tile([P, 1], F32, tag="bexp")
            nc.vector.memset(BEXP[:], LNC - KAPPA)
            BLN0 = cpool.tile([1, 1], F32, tag="bln0")
            nc.vector.memset(BLN0[:], 0.0)
            # identities built on device (no DMA)
            EYE = cpool.tile([P, NCH * N], F32, tag="eye")
            nc.vector.memset(EYE[:], 1.0)
            nc.gpsimd.affine_select(
                EYE[:].rearrange("p (c n) -> p c n", c=NCH),
                EYE[:].rearrange("p (c n) -> p c n", c=NCH),
                pattern=[[-P, NCH], [1, N]], compare_op=OP.is_equal,
                fill=0.0, base=0, channel_multiplier=-1,
            )
            E120S = cpool.tile([P, P], F32, tag="e120s")
            nc.vector.memset(E120S[:], 1.0)
            nc.gpsimd.affine_select(
                E120S[:], E120S[:], pattern=[[1, P]], compare_op=OP.is_equal,
                fill=0.0, base=0, channel_multiplier=-1,
            )
            E120 = cpool.tile([P, P], F32R, tag="e120")
            nc.vector.tensor_copy(E120[:], E120S[:])

            W = N + BL  # merged chunk width: [M_c | Q_c]

            def mm_group(ps, lhsT_tile, rhs_tile, i, rhs_w, rhs_stride=None):
                rs = rhs_w if rhs_stride is None else rhs_stride
                for c in range(NCH):
                    nc.tensor.matmul(
                        ps[:],
                        lhsT=lhsT_tile[:, c * N + i * P: c * N + i * P + P],
                        rhs=rhs_tile[:, c * rs: c * rs + rhs_w],
                        start=(c == 0), stop=(c == NCH - 1),
                    )

            def copy_out(dst_ap, ps, idx, small=False):
                # PSUM->SBUF copies: big 1/4 DVE 3/4 ACT; small (transpose
                # blocks) 1/2-1/2 -- ACT's fixed overhead dominates there
                mod = 2 if small else 4
                if idx % mod == 0:
                    nc.vector.tensor_copy(dst_ap, ps[:])
                else:
                    nc.scalar.copy(dst_ap, ps[:])

            # ---- p0 (von Mises) in Q-layout [P, NCH*BL] -------------------
            Q = qpool.tile([P, NCH * BL], F32R, tag="q")
            for c in range(NCH):
                qs = Q[:, c * BL:(c + 1) * BL]
                dl = tpool.tile([P, BL], F32, tag="t0")
                # delta = init - c_mesh  (cos is even, sign irrelevant)
                nc.vector.tensor_scalar(dl[:], IREP[:], CM[:, c:c + 1], None,
                                        op0=OP.subtract)
                ab = tpool.tile([P, BL], F32, tag="t1")
                nc.scalar.activation(ab[:], dl[:], AF.Abs)
                fl = tpool.tile([P, BL], F32, tag="t2")
                nc.vector.tensor_scalar(fl[:], ab[:], -1.0, TWO_PI,
                                        op0=OP.mult, op1=OP.add)
                w = tpool.tile([P, BL], F32, tag="t3")
                nc.vector.tensor_tensor(w[:], ab[:], fl[:], op=OP.min)
                u = tpool.tile([P, BL], F32, tag="t0")
                nc.vector.tensor_tensor(u[:], w[:], w[:], op=OP.mult)
                h = tpool.tile([P, BL], F32, tag="t1")
                nc.vector.tensor_scalar(h[:], u[:], COS_COEF[8], COS_COEF[7],
                                        op0=OP.mult, op1=OP.add)
                heng = nc.gpsimd if c == 1 else nc.vector
                for k in range(6, -1, -1):
                    heng.tensor_tensor(h[:], h[:], u[:], op=OP.mult)
                    heng.tensor_scalar(h[:], h[:], COS_COEF[k], None,
                                       op0=OP.add)
                # p0 = exp(kappa*cos - kappa + lnC)
                nc.scalar.activation(qs, h[:], AF.Exp, bias=BEXP[:],
                                     scale=KAPPA)

            # ---- residual Taylor on p0 (commutes with the bit applies):
            # V = Q + rdk_k*(X V), k=deg_r..1.  Steps are emitted interleaved
            # with the prelude/chain so the small matmuls fill PE bubbles.
            taylor_state = {"V": Q, "k": deg_r, "dst": None}

            def taylor_step():
                k = taylor_state["k"]
                if k < 1:
                    return
                Vc = taylor_state["V"]
                last = k == 1 and taylor_state["dst"] is not None
                Vn = None if last else vpool.tile([P, NCH * BL], F32R,
                                                  tag="V")
                for i in range(NCH):
                    ps = pss.tile([P, BL], F32, tag="ap")
                    mm_group(ps, XT, Vc, i, BL)
                    if last:
                        # final step writes straight into the MQ tile's Q
                        # slots -- keeps the install copies off the
                        # prelude->chain critical path
                        vs = taylor_state["dst"][:, i * W + N: (i + 1) * W]
                    else:
                        vs = Vn[:, i * BL:(i + 1) * BL]
                    nc.vector.tensor_tensor(
                        vs, ps[:], RDK[:, (k - 1) * BL: k * BL], op=OP.mult)
                    nc.gpsimd.tensor_tensor(
                        vs, vs, Q[:, i * BL:(i + 1) * BL], op=OP.add)
                taylor_state["V"] = Vn
                taylor_state["k"] = k - 1

            # ---- prelude: ascending Taylor S = I + sum X^k/k! -------------
            # critical path per step: 9 matmuls + 3 scale-copies; the S+=T
            # adds run on gpsimd off the PE path.  M tiles are MQ-shaped
            # ([M_c | Q_c] per chunk) so later bit-applies merge into the
            # squaring matmuls as 32 extra moving columns.
            S = mpool.tile([P, NCH * W], F32R, tag="M")
            taylor_state["dst"] = S
            for c in range(NCH):
                nc.vector.tensor_tensor(S[:, c * W: c * W + N],
                                        XN[:, c * N:(c + 1) * N],
                                        EYE[:, c * N:(c + 1) * N], op=OP.add)
            T = XN
            for k in range(2, deg_p + 1):
                Tn = mpool.tile([P, NCH * N], F32R, tag="T")
                for i in range(NCH):
                    ps = psb.tile([P, N], F32, tag="sq")
                    mm_group(ps, XT, T, i, N)
                    if i % 2 == 0:
                        nc.vector.tensor_scalar(Tn[:, i * N:(i + 1) * N],
                                                ps[:], 1.0 / k, None,
                                                op0=OP.mult)
                    else:
                        nc.scalar.mul(Tn[:, i * N:(i + 1) * N], ps[:], 1.0 / k)
                    seng = nc.gpsimd if i == 1 else nc.vector
                    seng.tensor_tensor(S[:, i * W: i * W + N],
                                       S[:, i * W: i * W + N],
                                       Tn[:, i * N:(i + 1) * N],
                                       op=OP.add)
                T = Tn
                taylor_step()

            ST = mpool.tile([P, NCH * N], F32R, tag="MT")

            def transpose_mq(MTt, Mt):
                k = 0
                for ib in range(NCH):
                    for cp in range(NCH):
                        pst = psb.tile([P, P], F32, tag="sq")
                        nc.tensor.transpose(
                            pst[:].bitcast(F32R),
                            Mt[:, ib * W + cp * P: ib * W + cp * P + P],
                            E120[:],
                        )
                        copy_out(MTt[:, cp * N + ib * P: cp * N + ib * P + P],
                                 pst, k, small=True)
                        k += 1

            transpose_mq(ST, S)
            M, MT = S, ST

            def square(Mc, MTc, bit=None):
                # Sn = Mc@Mc; if bit is not None also compute Mc@Q (merged
                # columns) and blend it into Sn's Q slot under the bit mask
                # (arithmetic blend: copy_predicated can't produce fp32r).
                Sn = mpool.tile([P, NCH * W], F32R, tag="M")
                STn = mpool.tile([P, NCH * N], F32R, tag="MT")
                wid = N if bit is None else W
                for i in range(NCH):
                    ps = psb.tile([P, wid], F32, tag="sq")
                    mm_group(ps, MTc, Mc, i, wid, rhs_stride=W)
                    copy_out(Sn[:, i * W: i * W + N], ps[:, :N], i)
                    if bit is not None:
                        qold = Mc[:, i * W + N: (i + 1) * W]
                        dq = tpool.tile([P, BL], F32, tag="blend")
                        nc.vector.tensor_tensor(dq[:], ps[:, N:W], qold,
                                                op=OP.subtract)
                        nc.vector.tensor_tensor(
                            dq[:], dq[:], MSK[:, bit * BL:(bit + 1) * BL],
                            op=OP.mult)
                        nc.gpsimd.tensor_tensor(
                            Sn[:, i * W + N: (i + 1) * W], dq[:], qold,
                            op=OP.add)
                transpose_mq(STn, Sn)
                return Sn, STn

            # drain remaining taylor steps; the k==1 step lands the
            # evolved p0 directly in the MQ tile's Q slots
            while taylor_state["k"] >= 1:
                taylor_step()

            # ---- merged bit applies + chain squarings ---------------------
            # level j squares M (= expm(2^j T0 A)) and applies bit j of the
            # quantized delay to Q in the same matmul set.  The top TWO bits
            # need no further squaring: bit k-2 is a single apply of M_{k-2}
            # and bit k-1 a double apply (M_{k-1} Q = M_{k-2} (M_{k-2} Q)),
            # which is ~2x cheaper than materializing M_{k-1}.
            for j in range(k_bits - 2):
                M, MT = square(M, MT, bit=j)

            def apply_bit(q_rhs, rhs_stride, rhs_off, bit, blend_src):
                # psum[i] = M @ q ; if bit is not None blend into a fresh
                # Q tile under the bit mask, else return psum tiles
                aps = []
                for i in range(NCH):
                    ps = pss.tile([P, BL], F32, tag="ap")
                    for c in range(NCH):
                        nc.tensor.matmul(
                            ps[:],
                            lhsT=MT[:, c * N + i * P: c * N + i * P + P],
                            rhs=q_rhs[:, c * rhs_stride + rhs_off:
                                      c * rhs_stride + rhs_off + BL],
                            start=(c == 0), stop=(c == NCH - 1),
                        )
                    aps.append(ps)
                if bit is None:
                    return aps
                Qn = qpool.tile([P, NCH * BL], F32R, tag="qf")
                for i in range(NCH):
                    qold = (blend_src[i]
                            if isinstance(blend_src, list)
                            else blend_src[:, i * rhs_stride + rhs_off:
                                           i * rhs_stride + rhs_off + BL])
                    dq = tpool.tile([P, BL], F32, tag="blend")
                    nc.vector.tensor_tensor(dq[:], aps[i][:], qold,
                                            op=OP.subtract)
                    nc.vector.tensor_tensor(
                        dq[:], dq[:], MSK[:, bit * BL:(bit + 1) * BL],
                        op=OP.mult)
                    nc.gpsimd.tensor_tensor(
                        Qn[:, i * BL:(i + 1) * BL], dq[:], qold, op=OP.add)
                return Qn

            # bit k-2: single apply on the Q slots of the MQ tile
            Qf = apply_bit(M, W, N, k_bits - 2, M)
            # bit k-1: double apply of the same M
            y1ps = apply_bit(Qf, BL, 0, None, None)
            Y1 = vpool.tile([P, NCH * BL], F32R, tag="V")
            for i in range(NCH):
                copy_out(Y1[:, i * BL:(i + 1) * BL], y1ps[i], i, small=True)
            V = apply_bit(Y1, BL, 0, k_bits - 1, Qf)

            # ---- selection + loss terms -----------------------------------
            sel = psb.tile([1, BL], F32, tag="sq")
            for c in range(NCH):
                tmp = tpool.tile([P, BL], F32R, tag="t2")
                nc.vector.tensor_tensor(
                    tmp[:], V[:, c * BL:(c + 1) * BL],
                    OH[:, c * BL:(c + 1) * BL], op=OP.mult)
                nc.tensor.matmul(sel[:], lhsT=ONES[:], rhs=tmp[:],
                                 start=(c == 0), stop=(c == NCH - 1))
            # ln(relu(psel)+eps) via exponent/mantissa split: the HW Ln
            # table degrades for huge args (psel can reach ~1e20 in the
            # weak-diffusion regime), so compute ln(m) + e*ln2 with m in
            # [1,2), which keeps the table in its accurate range.
            I32 = mybir.dt.int32
            rl = tpool.tile([1, BL], F32, tag="r0")
            nc.vector.tensor_scalar(rl[:], sel[:], 0.0, EPS,
                                    op0=OP.max, op1=OP.add)
            xi = rl[:].bitcast(I32)
            et = tpool.tile([1, BL], I32, tag="r2")
            nc.vector.tensor_scalar(et[:], xi, 23, None,
                                    op0=OP.arith_shift_right)
            ef = tpool.tile([1, BL], F32, tag="r3")
            nc.vector.tensor_copy(ef[:], et[:])
            mi = tpool.tile([1, BL], I32, tag="r4")
            nc.vector.tensor_scalar(mi[:], xi, 0x007FFFFF, 0x3F800000,
                                    op0=OP.bitwise_and, op1=OP.bitwise_or)
            lnm = tpool.tile([1, BL], F32, tag="r5")
            nc.scalar.activation(lnm[:], mi[:].bitcast(F32), AF.Ln,
                                 bias=BLN0[:], scale=1.0)
            terms = tpool.tile([1, BL], F32, tag="r1")
            # ef holds the biased exponent; fold the -127*ln2 into the mult
            nc.vector.tensor_scalar(terms[:], ef[:], 0.6931471805599453,
                                    -88.02969193111305,
                                    op0=OP.mult, op1=OP.add)
            nc.vector.tensor_tensor(terms[:], terms[:], lnm[:], op=OP.add)
            nc.sync.dma_start(d_out[:], terms[:])

    nc.compile()
    return nc


def _host_prep(c_mesh, gtheta, sigma_diff, init_color, delay_t, report_color):
    """Host-side glue: operator assembly (replicating reference f32 ops),
    plan selection, and per-core index/bit/layout arrays."""
    f32 = np.float32
    c = np.asarray(c_mesh, dtype=f32)
    g = np.asarray(gtheta, dtype=f32)
    s = np.asarray(sigma_diff, dtype=f32)[0]
    init = np.asarray(init_color, dtype=f32)
    t = np.asarray(delay_t, dtype=f32)
    rep = np.asarray(report_color, dtype=f32)

    d = (c[1] - c[0]).astype(f32)
    eye = np.eye(N, dtype=f32)
    up = np.roll(eye, -1, axis=1)
    dn = np.roll(eye, 1, axis=1)
    D1 = ((up - dn) / (f32(2.0) * d)).astype(f32)
    D2 = ((up - f32(2.0) * eye + dn) / (d * d)).astype(f32)
    A = ((s ** f32(2.0)) / f32(2.0) * D2 - D1 * g[None, :]).astype(f32)

    anorm = np.abs(A.astype(np.float64)).sum(axis=1).max()
    k_bits, deg_p, deg_r = plan = _plan(anorm)
    T0 = T_MAX / (1 << k_bits)
    X = (A * f32(T0)).astype(f32)

    m = np.floor(t.astype(np.float64) / T0).astype(np.int64)
    m = np.clip(m, 0, (1 << k_bits) - 1)
    r = (t.astype(np.float64) - m * T0) / T0  # in X = T0*A units
    bits = ((m[:, None] >> np.arange(k_bits)[None, :]) & 1)     # [B, K]
    idx = np.argmin(np.abs(c[None, :] - rep[:, None]), axis=1)

    shared = {
        "x": X,
        "xt": np.ascontiguousarray(X.T),
        "cm": np.ascontiguousarray(c.reshape(NCH, P).T),
    }
    in_maps = []
    for core in range(NCORES):
        sl = slice(core * BL, (core + 1) * BL)
        irep = np.broadcast_to(init[sl][None, :], (P, BL)).astype(f32)
        msk = np.broadcast_to(
            bits[sl].T.reshape(1, k_bits * BL), (P, k_bits * BL)
        ).astype(f32)  # bit j at [j*BL:(j+1)*BL]
        rdk = np.empty((deg_r, BL), f32)
        for k in range(1, deg_r + 1):
            rdk[k - 1] = (r[sl] / k).astype(f32)
        rdk = np.broadcast_to(
            rdk.reshape(1, deg_r * BL), (P, deg_r * BL)).astype(f32)
        oh = np.zeros((NCH, P, BL), f32)
        for b, ix in enumerate(idx[sl]):
            oh[ix // P, ix % P, b] = 1.0
        oh = np.ascontiguousarray(oh.transpose(1, 0, 2).reshape(P, NCH * BL))
        in_maps.append(dict(shared, irep=irep, msk=msk, rdk=rdk, oh=oh))
    return plan, in_maps


def _get_nc(plan):
    if plan not in _COMPILED:
        _COMPILED[plan] = _build_bass(*plan)
    return _COMPILED[plan]


def kernel(**inputs):
    from concourse.bass_utils import run_bass_kernel_spmd

    plan, in_maps = _host_prep(
        inputs["c_mesh"], inputs["gtheta"], inputs["sigma_diff"],
        inputs["init_color"], inputs["delay_t"], inputs["report_color"],
    )
    nc = _get_nc(plan)
    res = run_bass_kernel_spmd(nc, in_maps, list(range(NCORES)))
    terms = np.concatenate(
        [np.asarray(res.results[k]["terms"]).reshape(-1) for k in range(NCORES)]
    )
    loss = -np.mean(terms.astype(np.float64))
    return np.asarray(loss, dtype=np.float32)



# revision 30
# speedup vs baseline: 1.8146x; 1.1232x over previous
"""Trainium2 Bass kernel for the circular drift-diffusion loss (batched expm).

Reference computes  loss = -mean_b log(relu(e_{idx_b}^T expm(t_b*A) p0_b) + eps)
with A a fixed 360x360 circular advection-diffusion operator, t_b in [0,1000),
p0_b a von Mises density, over a batch of 256.

Algorithm (per core; batch sharded 32/core over 8 cores):
  * Quantize t_b = m_b*T0 + r_b with T0 = 1000/2^K, m_b < 2^K.
  * Build propagator chain M_j = expm(2^j*T0*A) once by repeated squaring
    (prelude: ascending Taylor at T0/2^PRE_SQ, then PRE_SQ squarings -> M_0;
    then K-1 squarings).  A squaring is 9 f32 matmuls for S = M@M plus 9 PE
    transposes for S^T (needed as the next stationary operand).  K and the
    Taylor degrees are chosen at runtime from ||A||_inf so both
    heavy-diffusion and near-advection inputs are optimal.
  * Apply bits of m_b as masked batched matvecs: Q <- bit_j ? M_j Q : Q.
  * Residual: Q <- Taylor_DEG_R(r_b A) Q (Horner, per-sample scalar folded
    into host-precomputed r/k coefficient tables).
  * p0 built on device (folded poly cos + Exp activation), selection via
    one-hot + PE column-sum, loss terms via Ln activation.
Everything O(n^2)+ runs on device; host does only index/bit/layout glue and
the tridiagonal operator assembly (exactly replicating the reference's f32
evo_mat construction).
"""

import math

import numpy as np

# ---------------- static problem constants (hardcoded per contract) ----------
N = 360            # color mesh size
P = 120            # partition chunk (N = 3*P)
NCH = 3            # chunks
B = 256            # total batch
NCORES = 8
BL = B // NCORES   # per-core batch
T_MAX = 1000.0
KAPPA = 400.0      # 1/SIGMA_INIT^2
EPS = 1e-5
TWO_PI = 6.283185307179586
# ln(1/(2*pi*i0e(400)))  [i0e(400) = 0.019953356281939987]
LNC = 2.076480848703078
# cos(sqrt(u)) on u in [0, pi^2] (|delta| folded to [0,pi]), power basis c0..c8
COS_COEF = [1.00000000e+00, -5.00000000e-01, 4.16666666e-02, -1.38888885e-03,
            2.48015646e-05, -2.75566515e-07, 2.08651966e-09, -1.13535474e-11,
            4.13131734e-14]

_COMPILED = {}


def _taylor_deg(x, tol, lo):
    """Smallest d with x^(d+1)/(d+1)! < tol."""
    d = lo
    term = x ** (d + 1) / math.factorial(d + 1)
    while term > tol and d < 40:
        d += 1
        term *= x / (d + 1)
    return d


def _plan(anorm):
    """Choose (k_bits, deg_p, deg_r) from ||A||_inf.  The time grid is
    T0 = T_MAX/2^k_bits, chosen so the prelude Taylor converges fast at T0;
    every squaring level applies one bit of the quantized delay."""
    xa = T_MAX * float(anorm)
    if xa <= 0.0:
        return 2, 4, 3
    k0 = max(2, min(16, math.ceil(math.log2(max(xa / 0.9, 2.0)))))

    def degrees(k):
        x0 = xa / (1 << k)
        # prelude truncation amplifies roughly 2^(k/2) through the
        # squarings, so its tolerance adapts to chain depth; the residual
        # Taylor is applied once (no amplification) and tolerates more.
        tol_p = min(max(3e-5 / 2 ** (k / 2), 5e-8), 2e-6)
        return _taylor_deg(x0, tol_p, 4), _taylor_deg(x0, 1e-6, 3)

    # pick k by explicit cost minimization with measured per-stage costs
    # (chain level 7.6us, prelude step 4.5us, taylor step 0.7us)
    best = None
    for k in range(max(2, k0 - 1), min(16, k0 + 2) + 1):
        dp, dr = degrees(k)
        cost = (k - 1) * 7.6 + (dp - 1) * 4.5 + dr * 0.7
        if best is None or cost < best[0]:
            best = (cost, k, dp, dr)
    _, k, deg_p, deg_r = best
    return k, deg_p, deg_r


def _build_bass(k_bits, deg_p, deg_r):
    """Construct the Bass program (SPMD; identical on all 8 cores)."""
    import concourse.tile as tile
    from concourse import bacc, mybir

    F32 = mybir.dt.float32
    F32R = mybir.dt.float32r
    AF = mybir.ActivationFunctionType
    OP = mybir.AluOpType

    nc = bacc.Bacc("TRN2", target_bir_lowering=False, debug=False)

    def din(name, shape, dt=F32):
        return nc.dram_tensor(name, shape, dt, kind="ExternalInput").ap()

    d_x = din("x", [N, N], F32R)  # X = T0*A
    d_xt = din("xt", [N, N], F32R)  # X^T
    d_q0 = din("q0", [P, NCH * BL], F32R)    # p0 chunks (host von Mises)
    d_msk = din("msk", [P, k_bits * BL])     # bit masks (0.0/1.0)
    d_rdk = din("rdk", [P, deg_r * BL])      # (r/T0)/k, k=1..deg_r
    d_oh = din("oh", [P, NCH * BL])          # one-hot selector chunks
    d_out = nc.dram_tensor("terms", [1, BL], F32, kind="ExternalOutput").ap()

    with tile.TileContext(nc) as tc:
        with (
            tc.tile_pool(name="const", bufs=1) as cpool,
            tc.tile_pool(name="mats", bufs=3) as mpool,
            tc.tile_pool(name="qp", bufs=2) as qpool,
            tc.tile_pool(name="vp", bufs=3) as vpool,
            tc.tile_pool(name="tp", bufs=4) as tpool,
            tc.tile_pool(name="psb", bufs=5, space="PSUM") as psb,
            tc.tile_pool(name="pss", bufs=3, space="PSUM") as pss,
        ):
            # ---- constants ------------------------------------------------
            # chunk 0 of x/xt lands as [P,P] pieces on parallel queues so
            # the first prelude matmul starts early; other chunks whole
            XN = cpool.tile([P, NCH * N], F32R, tag="x")
            XT = cpool.tile([P, NCH * N], F32R, tag="xt")
            # chunk 0 of x lands as [P,P] pieces plus the first XT block so
            # the first prelude matmuls start early; other chunks whole
            for b in range(NCH):
                nc.sync.dma_start(XN[:, b * P:(b + 1) * P],
                                  d_x[0:P, b * P:(b + 1) * P])
            nc.sync.dma_start(XT[:, 0:P], d_xt[0:P, 0:P])
            for c in range(1, NCH):
                nc.sync.dma_start(XN[:, c * N:(c + 1) * N],
                                  d_x[c * P:(c + 1) * P, :])
                nc.sync.dma_start(XT[:, c * N:(c + 1) * N],
                                  d_xt[c * P:(c + 1) * P, :])
            nc.sync.dma_start(XT[:, P:N], d_xt[0:P, P:N])
            MSK = cpool.tile([P, k_bits * BL], F32, tag="msk")
            nc.sync.dma_start(MSK[:], d_msk[:])
            RDK = cpool.tile([P, deg_r * BL], F32, tag="rdk")
            nc.sync.dma_start(RDK[:], d_rdk[:])
            OH = cpool.tile([P, NCH * BL], F32, tag="oh")
            nc.sync.dma_start(OH[:], d_oh[:])
            # fp32r matmul operands must be produced as fp32r (walrus
            # verifier); memset can't, so constants go via a staging copy
            ONES0 = cpool.tile([P, 1], F32, tag="ones0")
            nc.vector.memset(ONES0[:], 1.0)
            ONES = cpool.tile([P, 1], F32R, tag="ones")
            nc.vector.tensor_copy(ONES[:], ONES0[:])
            BLN0 = cpool.tile([1, 1], F32, tag="bln0")
            nc.vector.memset(BLN0[:], 0.0)
            # identities built on device (no DMA)
            EYE = cpool.tile([P, NCH * N], F32, tag="eye")
            nc.vector.memset(EYE[:], 1.0)
            nc.gpsimd.affine_select(
                EYE[:].rearrange("p (c n) -> p c n", c=NCH),
                EYE[:].rearrange("p (c n) -> p c n", c=NCH),
                pattern=[[-P, NCH], [1, N]], compare_op=OP.is_equal,
                fill=0.0, base=0, channel_multiplier=-1,
            )
            E120S = cpool.tile([P, P], F32, tag="e120s")
            nc.vector.memset(E120S[:], 1.0)
            nc.gpsimd.affine_select(
                E120S[:], E120S[:], pattern=[[1, P]], compare_op=OP.is_equal,
                fill=0.0, base=0, channel_multiplier=-1,
            )
            E120 = cpool.tile([P, P], F32R, tag="e120")
            nc.vector.tensor_copy(E120[:], E120S[:])

            W = N + BL  # merged chunk width: [M_c | Q_c]

            def mm_group(ps, lhsT_tile, rhs_tile, i, rhs_w, rhs_stride=None):
                rs = rhs_w if rhs_stride is None else rhs_stride
                for c in range(NCH):
                    nc.tensor.matmul(
                        ps[:],
                        lhsT=lhsT_tile[:, c * N + i * P: c * N + i * P + P],
                        rhs=rhs_tile[:, c * rs: c * rs + rhs_w],
                        start=(c == 0), stop=(c == NCH - 1),
                    )

            def copy_out(dst_ap, ps, idx, small=False):
                # PSUM->SBUF copies: big 1/4 DVE 3/4 ACT; small (transpose
                # blocks) 1/2-1/2 -- ACT's fixed overhead dominates there
                mod = 2 if small else 4
                if idx % mod == 0:
                    nc.vector.tensor_copy(dst_ap, ps[:])
                else:
                    nc.scalar.copy(dst_ap, ps[:])

            # ---- p0 (von Mises) DMA'd from host in Q-layout [P, NCH*BL] ---
            Q = qpool.tile([P, NCH * BL], F32R, tag="q")
            nc.sync.dma_start(Q[:], d_q0[:])

            # ---- residual Taylor on p0 (commutes with the bit applies):
            # V = Q + rdk_k*(X V), k=deg_r..1.  Steps are emitted interleaved
            # with the prelude/chain so the small matmuls fill PE bubbles.
            taylor_state = {"V": Q, "k": deg_r, "dst": None}

            def taylor_step():
                k = taylor_state["k"]
                if k < 1:
                    return
                Vc = taylor_state["V"]
                last = k == 1 and taylor_state["dst"] is not None
                Vn = None if last else vpool.tile([P, NCH * BL], F32R,
                                                  tag="V")
                for i in range(NCH):
                    ps = pss.tile([P, BL], F32, tag="ap")
                    mm_group(ps, XT, Vc, i, BL)
                    if last:
                        # final step writes straight into the MQ tile's Q
                        # slots -- keeps the install copies off the
                        # prelude->chain critical path
                        vs = taylor_state["dst"][:, i * W + N: (i + 1) * W]
                    else:
                        vs = Vn[:, i * BL:(i + 1) * BL]
                    nc.vector.tensor_tensor(
                        vs, ps[:], RDK[:, (k - 1) * BL: k * BL], op=OP.mult)
                    nc.gpsimd.tensor_tensor(
                        vs, vs, Q[:, i * BL:(i + 1) * BL], op=OP.add)
                taylor_state["V"] = Vn
                taylor_state["k"] = k - 1

            # ---- prelude: ascending Taylor S = I + sum X^k/k! -------------
            # critical path per step: 9 matmuls + 3 scale-copies; the S+=T
            # adds run on gpsimd off the PE path.  M tiles are MQ-shaped
            # ([M_c | Q_c] per chunk) so later bit-applies merge into the
            # squaring matmuls as 32 extra moving columns.
            S = mpool.tile([P, NCH * W], F32R, tag="M")
            taylor_state["dst"] = S
            for c in range(NCH):
                nc.vector.tensor_tensor(S[:, c * W: c * W + N],
                                        XN[:, c * N:(c + 1) * N],
                                        EYE[:, c * N:(c + 1) * N], op=OP.add)
            T = XN
            for k in range(2, deg_p + 1):
                Tn = mpool.tile([P, NCH * N], F32R, tag="T")
                for i in range(NCH):
                    ps = psb.tile([P, N], F32, tag="sq")
                    mm_group(ps, XT, T, i, N)
                    if i % 2 == 0:
                        nc.vector.tensor_scalar(Tn[:, i * N:(i + 1) * N],
                                                ps[:], 1.0 / k, None,
                                                op0=OP.mult)
                    else:
                        nc.scalar.mul(Tn[:, i * N:(i + 1) * N], ps[:], 1.0 / k)
                    seng = nc.gpsimd if i == 1 else nc.vector
                    seng.tensor_tensor(S[:, i * W: i * W + N],
                                       S[:, i * W: i * W + N],
                                       Tn[:, i * N:(i + 1) * N],
                                       op=OP.add)
                T = Tn
                taylor_step()

            ST = mpool.tile([P, NCH * N], F32R, tag="MT")

            def transpose_mq(MTt, Mt):
                k = 0
                for ib in range(NCH):
                    for cp in range(NCH):
                        pst = psb.tile([P, P], F32, tag="sq")
                        nc.tensor.transpose(
                            pst[:].bitcast(F32R),
                            Mt[:, ib * W + cp * P: ib * W + cp * P + P],
                            E120[:],
                        )
                        copy_out(MTt[:, cp * N + ib * P: cp * N + ib * P + P],
                                 pst, k, small=True)
                        k += 1

            transpose_mq(ST, S)
            M, MT = S, ST

            def square(Mc, MTc, bit=None):
                # Sn = Mc@Mc; if bit is not None also compute Mc@Q (merged
                # columns) and blend it into Sn's Q slot under the bit mask
                # (arithmetic blend: copy_predicated can't produce fp32r).
                Sn = mpool.tile([P, NCH * W], F32R, tag="M")
                STn = mpool.tile([P, NCH * N], F32R, tag="MT")
                wid = N if bit is None else W
                for i in range(NCH):
                    ps = psb.tile([P, wid], F32, tag="sq")
                    mm_group(ps, MTc, Mc, i, wid, rhs_stride=W)
                    copy_out(Sn[:, i * W: i * W + N], ps[:, :N], i)
                    if bit is not None:
                        qold = Mc[:, i * W + N: (i + 1) * W]
                        dq = tpool.tile([P, BL], F32, tag="blend")
                        nc.vector.tensor_tensor(dq[:], ps[:, N:W], qold,
                                                op=OP.subtract)
                        nc.vector.tensor_tensor(
                            dq[:], dq[:], MSK[:, bit * BL:(bit + 1) * BL],
                            op=OP.mult)
                        nc.gpsimd.tensor_tensor(
                            Sn[:, i * W + N: (i + 1) * W], dq[:], qold,
                            op=OP.add)
                transpose_mq(STn, Sn)
                return Sn, STn

            # drain remaining taylor steps; the k==1 step lands the
            # evolved p0 directly in the MQ tile's Q slots
            while taylor_state["k"] >= 1:
                taylor_step()

            # ---- merged bit applies + chain squarings ---------------------
            # level j squares M (= expm(2^j T0 A)) and applies bit j of the
            # quantized delay to Q in the same matmul set.  The top TWO bits
            # need no further squaring: bit k-2 is a single apply of M_{k-2}
            # and bit k-1 a double apply (M_{k-1} Q = M_{k-2} (M_{k-2} Q)),
            # which is ~2x cheaper than materializing M_{k-1}.
            for j in range(k_bits - 2):
                M, MT = square(M, MT, bit=j)

            def apply_bit(q_rhs, rhs_stride, rhs_off, bit, blend_src):
                # psum[i] = M @ q ; if bit is not None blend into a fresh
                # Q tile under the bit mask, else return psum tiles
                aps = []
                for i in range(NCH):
                    ps = pss.tile([P, BL], F32, tag="ap")
                    for c in range(NCH):
                        nc.tensor.matmul(
                            ps[:],
                            lhsT=MT[:, c * N + i * P: c * N + i * P + P],
                            rhs=q_rhs[:, c * rhs_stride + rhs_off:
                                      c * rhs_stride + rhs_off + BL],
                            start=(c == 0), stop=(c == NCH - 1),
                        )
                    aps.append(ps)
                if bit is None:
                    return aps
                Qn = qpool.tile([P, NCH * BL], F32R, tag="qf")
                for i in range(NCH):
                    qold = (blend_src[i]
                            if isinstance(blend_src, list)
                            else blend_src[:, i * rhs_stride + rhs_off:
                                           i * rhs_stride + rhs_off + BL])
                    dq = tpool.tile([P, BL], F32, tag="blend")
                    nc.vector.tensor_tensor(dq[:], aps[i][:], qold,
                                            op=OP.subtract)
                    nc.vector.tensor_tensor(
                        dq[:], dq[:], MSK[:, bit * BL:(bit + 1) * BL],
                        op=OP.mult)
                    nc.gpsimd.tensor_tensor(
                        Qn[:, i * BL:(i + 1) * BL], dq[:], qold, op=OP.add)
                return Qn

            # bit k-2: single apply on the Q slots of the MQ tile
            Qf = apply_bit(M, W, N, k_bits - 2, M)
            # bit k-1: double apply of the same M
            y1ps = apply_bit(Qf, BL, 0, None, None)
            Y1 = vpool.tile([P, NCH * BL], F32R, tag="V")
            for i in range(NCH):
                copy_out(Y1[:, i * BL:(i + 1) * BL], y1ps[i], i, small=True)
            V = apply_bit(Y1, BL, 0, k_bits - 1, Qf)

            # ---- selection + loss terms -----------------------------------
            sel = psb.tile([1, BL], F32, tag="sq")
            for c in range(NCH):
                tmp = tpool.tile([P, BL], F32R, tag="t2")
                nc.vector.tensor_tensor(
                    tmp[:], V[:, c * BL:(c + 1) * BL],
                    OH[:, c * BL:(c + 1) * BL], op=OP.mult)
                nc.tensor.matmul(sel[:], lhsT=ONES[:], rhs=tmp[:],
                                 start=(c == 0), stop=(c == NCH - 1))
            # ln(relu(psel)+eps) via exponent/mantissa split: the HW Ln
            # table degrades for huge args (psel can reach ~1e20 in the
            # weak-diffusion regime), so compute ln(m) + e*ln2 with m in
            # [1,2), which keeps the table in its accurate range.
            I32 = mybir.dt.int32
            rl = tpool.tile([1, BL], F32, tag="r0")
            nc.vector.tensor_scalar(rl[:], sel[:], 0.0, EPS,
                                    op0=OP.max, op1=OP.add)
            xi = rl[:].bitcast(I32)
            et = tpool.tile([1, BL], I32, tag="r2")
            nc.vector.tensor_scalar(et[:], xi, 23, None,
                                    op0=OP.arith_shift_right)
            ef = tpool.tile([1, BL], F32, tag="r3")
            nc.vector.tensor_copy(ef[:], et[:])
            mi = tpool.tile([1, BL], I32, tag="r4")
            nc.vector.tensor_scalar(mi[:], xi, 0x007FFFFF, 0x3F800000,
                                    op0=OP.bitwise_and, op1=OP.bitwise_or)
            lnm = tpool.tile([1, BL], F32, tag="r5")
            nc.scalar.activation(lnm[:], mi[:].bitcast(F32), AF.Ln,
                                 bias=BLN0[:], scale=1.0)
            terms = tpool.tile([1, BL], F32, tag="r1")
            # ef holds the biased exponent; fold the -127*ln2 into the mult
            nc.vector.tensor_scalar(terms[:], ef[:], 0.6931471805599453,
                                    -88.02969193111305,
                                    op0=OP.mult, op1=OP.add)
            nc.vector.tensor_tensor(terms[:], terms[:], lnm[:], op=OP.add)
            nc.sync.dma_start(d_out[:], terms[:])

    nc.compile()
    return nc


def _host_prep(c_mesh, gtheta, sigma_diff, init_color, delay_t, report_color):
    """Host-side glue: operator assembly (replicating reference f32 ops),
    plan selection, and per-core index/bit/layout arrays."""
    f32 = np.float32
    c = np.asarray(c_mesh, dtype=f32)
    g = np.asarray(gtheta, dtype=f32)
    s = np.asarray(sigma_diff, dtype=f32)[0]
    init = np.asarray(init_color, dtype=f32)
    t = np.asarray(delay_t, dtype=f32)
    rep = np.asarray(report_color, dtype=f32)

    d = (c[1] - c[0]).astype(f32)
    eye = np.eye(N, dtype=f32)
    up = np.roll(eye, -1, axis=1)
    dn = np.roll(eye, 1, axis=1)
    D1 = ((up - dn) / (f32(2.0) * d)).astype(f32)
    D2 = ((up - f32(2.0) * eye + dn) / (d * d)).astype(f32)
    A = ((s ** f32(2.0)) / f32(2.0) * D2 - D1 * g[None, :]).astype(f32)

    anorm = np.abs(A.astype(np.float64)).sum(axis=1).max()
    k_bits, deg_p, deg_r = plan = _plan(anorm)
    T0 = T_MAX / (1 << k_bits)
    X = (A * f32(T0)).astype(f32)

    m = np.floor(t.astype(np.float64) / T0).astype(np.int64)
    m = np.clip(m, 0, (1 << k_bits) - 1)
    r = (t.astype(np.float64) - m * T0) / T0  # in X = T0*A units
    bits = ((m[:, None] >> np.arange(k_bits)[None, :]) & 1)     # [B, K]
    idx = np.argmin(np.abs(c[None, :] - rep[:, None]), axis=1)

    # p0 host-side (O(B*n) glue, like the one-hot/argmin prep): von Mises
    # density replicating the reference's f32 formula
    z = np.cos(c[None, :].astype(np.float64)
               - init[:, None].astype(np.float64)) - 1.0
    p0 = (np.exp(KAPPA * z + LNC)).astype(f32)          # [B, n]

    shared = {
        "x": X,
        "xt": np.ascontiguousarray(X.T),
    }
    in_maps = []
    for core in range(NCORES):
        sl = slice(core * BL, (core + 1) * BL)
        # Q layout [P, NCH*BL]: chunk c at cols [c*BL:(c+1)*BL], Q[p,c*BL+b]
        # = p0[b, c*P+p]
        q0 = np.ascontiguousarray(
            p0[sl].reshape(BL, NCH, P).transpose(2, 1, 0).reshape(P, NCH * BL))
        msk = np.broadcast_to(
            bits[sl].T.reshape(1, k_bits * BL), (P, k_bits * BL)
        ).astype(f32)  # bit j at [j*BL:(j+1)*BL]
        rdk = np.empty((deg_r, BL), f32)
        for k in range(1, deg_r + 1):
            rdk[k - 1] = (r[sl] / k).astype(f32)
        rdk = np.broadcast_to(
            rdk.reshape(1, deg_r * BL), (P, deg_r * BL)).astype(f32)
        oh = np.zeros((NCH, P, BL), f32)
        for b, ix in enumerate(idx[sl]):
            oh[ix // P, ix % P, b] = 1.0
        oh = np.ascontiguousarray(oh.transpose(1, 0, 2).reshape(P, NCH * BL))
        in_maps.append(dict(shared, q0=q0, msk=msk, rdk=rdk, oh=oh))
    return plan, in_maps


def _get_nc(plan):
    if plan not in _COMPILED:
        _COMPILED[plan] = _build_bass(*plan)
    return _COMPILED[plan]


def kernel(**inputs):
    from concourse.bass_utils import run_bass_kernel_spmd

    plan, in_maps = _host_prep(
        inputs["c_mesh"], inputs["gtheta"], inputs["sigma_diff"],
        inputs["init_color"], inputs["delay_t"], inputs["report_color"],
    )
    nc = _get_nc(plan)
    res = run_bass_kernel_spmd(nc, in_maps, list(range(NCORES)))
    terms = np.concatenate(
        [np.asarray(res.results[k]["terms"]).reshape(-1) for k in range(NCORES)]
    )
    loss = -np.mean(terms.astype(np.float64))
    return np.asarray(loss, dtype=np.float32)

